# revision 17
# baseline (speedup 1.0000x reference)
"""Adaptive Huber/MSE/L1 loss on 8 TRN2 NeuronCores (Bass/Tile), v7.

Reference math (per sample, N = 4,096,000 elements):
    e   = pred - true
    L2  = mean(e^2);  L1 = mean(|e|)
    huber = (S2 - SR) * 0.5 / N     (S2 = sum e^2, SR = sum relu(|e|-5)^2)
    use_l2 = (L2 <= 1) | (L2 < L1^2)
    loss = mean_over_batch(where(use_l2, L2, huber))

Sharding: data-parallel, sample i -> core i. The host interleaves pred
and true at DMA-tile granularity into ONE [128, 64000] f32 DRAM tensor
per core (one dma_start / one completion semaphore per tile). Each core
emits a [1,48] f32 row of partial sums in S2 | SR | S1 column regions;
the host sums the regions and finishes the branch math during unshard.

Measured facts driving this layout (v4-v6 HW traces, this session):
  - Stream runs at ~430 GB/s = ~98% of the 435 GB/s SBUF-AXI fabric
    ceiling; 32.77 MB arrives over [5.2us, 86.5us]. That is the floor.
  - v6's straggler was DVE: sub (1.12ns/col, f32 in) + u16 |e| mask
    (0.34) + max (0.34) + dve-tile m^2 mults summed to ~75us of DVE
    work against an 82us stream - both DVE and ACT hit the tail ~6us
    behind and the collapse matmul ran at 93.7us. v7 cuts DVE's global
    load: ACT's Abs pass (paired across two tiles) produces |e| AND
    row-accumulates S1 in one instruction, so DVE drops the mask and
    PE drops the whole S1 PSUM chain (and its [1,500] reduce). DVE
    keeps sub + max(|e|,5)-5 + m^2 mult (pair-wide ops), PE row-sums
    all m^2 chunks into one d2 chain.
  - ACT cost per pass ~0.975ns/col + 0.28us ACCUM_READ + ~0.18us gap
    per instruction; pairing passes across two tiles halves the fixed
    costs. Pairing is main-body only - in the tail it delays the pass
    until the later tile's sub and hurts latency.
  - The ~250-instruction per-semaphore NEFF epilogue (~6us) and ~5us
    preamble are compiler-generated and fixed (queue-trim fails NRT).
  - tensor_tensor_reduce and all GpSimd tensor ops die at compile or
    NRT exec on this build; DVE tensor_scalar rejects abs_max. Probed.

Tail tiles shrink 1500/1000/800/500/200; the 200-col last tile runs a
short DVE-local chain (mask variant, no cross-engine hop), ACT squares
it and reduces the d2 PSUM bank via Identity+accum, then one fp32
partition-collapse matmul and a 192 B output DMA finish the kernel.
"""

import numpy as np

import concourse.bass as bass
import concourse.bacc as bacc
import concourse.mybir as mybir
from concourse.tile import TileContext
from concourse.bass_utils import run_bass_kernel_spmd

P = 128
COLS = 32000  # 160*160*160 / 128
DELTA = 5.0
N_CORES = 8
N_ELEM = float(P * COLS)
CHUNK = 500  # PE reduction column-chunk (PSUM bank limit 512 f32)
NF = 48  # fin columns: [0,16)=S2, [16,32)=SR, [32,48)=S1

F32 = mybir.dt.float32
U16 = mybir.dt.uint16
BF16 = mybir.dt.bfloat16
ALU = mybir.AluOpType
ACTF = mybir.ActivationFunctionType
AX = mybir.AxisListType

TILES = [2000] * 14 + [1500, 1000, 800, 500, 200]
LAST = len(TILES) - 1  # t18: short DVE-local final tile
PAIRS = [(0, 1), (2, 3), (4, 5), (6, 7), (8, 9), (10, 11), (12, 13)]
TAIL_SINGLE = {14, 15, 16, 17}  # per-tile ACT Abs+Square, no pairing

# fin columns: one per ACT accum / DVE reduce
S2_COL = {p: i for i, p in enumerate(PAIRS)}
S2_COL.update({14: 7, 15: 8, 16: 9, 17: 10, 18: 11})
SR_D2 = 16  # d2 PSUM chain Identity-reduce
SR_T18 = 17
S1_COL = {p: 32 + i for i, p in enumerate(PAIRS)}
S1_COL.update({14: 39, 15: 40, 16: 41, 17: 42, 18: 43})


def build():
    assert sum(TILES) == COLS
    partner = {}
    for a, b in PAIRS:
        partner[a] = (a, b)
        partner[b] = (a, b)

    # d2 chain: m^2 chunks for every tile except the last
    mm_d2 = sum(
        (w + CHUNK - 1) // CHUNK for t, w in enumerate(TILES) if t != LAST
    )

    nc = bacc.Bacc(
        "TRN2",
        target_bir_lowering=False,
        debug=False,
        enable_asserts=False,
        num_devices=N_CORES,
    )
    x_ext = nc.dram_tensor("x", [P, 2 * COLS], F32, kind="ExternalInput")
    out_ext = nc.dram_tensor("out", [1, NF], F32, kind="ExternalOutput")

    with TileContext(nc) as tc:
        with (
            tc.tile_pool(name="io", bufs=5) as io_pool,
            tc.tile_pool(name="work", bufs=3) as work_pool,
            tc.tile_pool(name="acc", bufs=1) as acc_pool,
            tc.tile_pool(name="psum", bufs=1, space="PSUM") as psum_pool,
        ):
            fin = acc_pool.tile([P, NF], F32)
            fin2 = acc_pool.tile([1, NF], F32)
            scr = acc_pool.tile([P, 4000], BF16)  # ACT Square output sink
            ones_bf = acc_pool.tile([P, 1], BF16)
            ones_f = acc_pool.tile([P, 1], F32)
            nc.vector.memset(ones_bf[:], 1.0)
            nc.vector.memset(ones_f[:], 1.0)
            nc.vector.memset(fin[:], 0.0)
            psum_d2 = psum_pool.tile([1, CHUNK], F32)  # m^2 chain t0..17
            ps2 = psum_pool.tile([1, NF], F32)

            io_tiles = []
            col = 0
            for t, w in enumerate(TILES):
                xt = io_pool.tile([P, 2 * w], F32, tag="x")
                nc.sync.dma_start(out=xt[:], in_=x_ext[:, 2 * col : 2 * col + 2 * w])
                io_tiles.append(xt)
                col += w
            assert col == COLS

            # raw-e / |e| / m buffers; pairs share one buffer so ACT's
            # Abs and Square passes (and DVE's max / m^2 mult) each run
            # once per pair. Allocate in USAGE order: pool ring slots
            # chain WAR deps in allocation order, so upfront allocation
            # makes an early tile wait on a later tile's readers
            # (measured: stream collapse at 80us in v6).
            e_bufs = {}
            a_bufs = {}
            m_bufs = {}

            mmd_i = 0
            for t, w in enumerate(TILES):
                if t not in e_bufs:
                    if t in partner:
                        pa, pb = partner[t]
                        pw = TILES[pa] + TILES[pb]
                        eb = work_pool.tile([P, pw], BF16, tag="e", name=f"ep{pa}")
                        ab = work_pool.tile([P, pw], BF16, tag="a", name=f"ap{pa}")
                        mb = work_pool.tile([P, pw], BF16, tag="m", name=f"mp{pa}")
                        e_bufs[pa] = (eb, 0)
                        e_bufs[pb] = (eb, TILES[pa])
                        a_bufs[pa] = (ab, 0)
                        m_bufs[pa] = (mb, 0)
                    else:
                        eb = work_pool.tile([P, w], BF16, tag="e", name=f"es{t}")
                        e_bufs[t] = (eb, 0)
                        if t != LAST:
                            ab = work_pool.tile([P, w], BF16, tag="a", name=f"as{t}")
                            a_bufs[t] = (ab, 0)
                        mb = work_pool.tile([P, w], BF16, tag="m", name=f"ms{t}")
                        m_bufs[t] = (mb, 0)
                xt = io_tiles[t]
                ebuf, eo = e_bufs[t]
                e = ebuf[:, eo : eo + w]
                # e = a - b (bf16 out: unbiased rounding, ~1e-5 rel err
                # on the final loss, far under the 2e-2 gate)
                nc.vector.tensor_tensor(e, xt[:, 0:w], xt[:, w : 2 * w], ALU.subtract)

                if t in partner:
                    pa, pb = partner[t]
                    if t != pb:
                        continue  # pair ops run once, at the later tile
                    pw = TILES[pa] + TILES[pb]
                    key = (pa, pb)
                    epair = e_bufs[pa][0][:, 0:pw]
                    apair = a_bufs[pa][0][:, 0:pw]
                    mpair = m_bufs[pa][0][:, 0:pw]
                elif t != LAST:
                    pw = w
                    key = t
                    epair = e
                    apair = a_bufs[t][0][:, 0:w]
                    mpair = m_bufs[t][0][:, 0:w]
                else:
                    # t18: DVE-local, no cross-engine |e| round trip
                    eu = ebuf.bitcast(U16)[:, eo : eo + w]
                    m = m_bufs[t][0][:, 0:w]
                    nc.vector.tensor_scalar(eu, eu, 0x7FFF, None, ALU.bitwise_and)
                    nc.scalar.activation(  # e^2 = |e|^2, off DVE's chain
                        scr[:, 0:w], e, ACTF.Square,
                        accum_out=fin[:, S2_COL[t] : S2_COL[t] + 1],
                    )
                    nc.vector.tensor_reduce(
                        fin[:, S1_COL[t] : S1_COL[t] + 1], e, axis=AX.X,
                        op=ALU.add, apply_absolute_value=True,
                    )
                    nc.vector.tensor_scalar(m, e, DELTA, -DELTA, ALU.max, ALU.add)
                    nc.vector.tensor_tensor(m, m, m, ALU.mult)
                    nc.vector.tensor_reduce(
                        fin[:, SR_T18 : SR_T18 + 1], m, axis=AX.X, op=ALU.add
                    )
                    continue

                # ACT: |e| -> apair with S1 row-accumulate, e^2 -> scr
                # with S2 row-accumulate (reads raw e, no mask needed)
                nc.scalar.activation(
                    apair, epair, ACTF.Abs,
                    accum_out=fin[:, S1_COL[key] : S1_COL[key] + 1],
                )
                nc.scalar.activation(
                    scr[:, 0:pw], epair, ACTF.Square,
                    accum_out=fin[:, S2_COL[key] : S2_COL[key] + 1],
                )
                # DVE: m = max(|e|,5)-5, then m^2 in place
                nc.vector.tensor_scalar(
                    mpair, apair, DELTA, -DELTA, ALU.max, ALU.add
                )
                nc.vector.tensor_tensor(mpair, mpair, mpair, ALU.mult)
                # PE: row-sum m^2 chunks into the d2 chain
                nch = (pw + CHUNK - 1) // CHUNK
                mb0 = mpair
                for c in range(nch):
                    cw = min(CHUNK, pw - c * CHUNK)
                    nc.tensor.matmul(
                        psum_d2[0:1, 0:cw], ones_bf[:, 0:1],
                        mb0[:, c * CHUNK : c * CHUNK + cw],
                        start=(mmd_i == 0), stop=(mmd_i == mm_d2 - 1),
                    )
                    mmd_i += 1
            assert mmd_i == mm_d2

            # d2 chain [1,500] reduce on ACT (Identity + accumulator);
            # chain closed at t17's PE chunk, ACT reaches this after its
            # tail passes - parallel with DVE's t18 chain
            nc.scalar.activation(
                scr[0:1, 0:CHUNK], psum_d2[0:1, :], ACTF.Identity,
                accum_out=fin[0:1, SR_D2 : SR_D2 + 1],
            )

            # partition-collapse so the output is one 192 B DMA packet
            nc.tensor.matmul(ps2[0:1, 0:NF], ones_f[:, 0:1], fin[:, 0:NF],
                             start=True, stop=True)
            nc.vector.tensor_scalar(fin2[:], ps2[0:1, 0:NF], 1.0, None, ALU.mult)
            nc.sync.dma_start(out=out_ext[:, :], in_=fin2[:])

    nc.compile()
    return nc


_NC_CACHE = {}


def _get_nc():
    if "nc" not in _NC_CACHE:
        _NC_CACHE["nc"] = build()
    return _NC_CACHE["nc"]


def _pack(a: np.ndarray, b: np.ndarray) -> np.ndarray:
    """Interleave pred/true at DMA-tile granularity: one [P, 2*COLS]
    tensor per core, tile t occupying cols [2*off, 2*off+2*w) with the
    pred block first and the true block second."""
    x = np.empty((N_CORES, P, 2 * COLS), dtype=np.float32)
    off = 0
    for w in TILES:
        x[:, :, 2 * off : 2 * off + w] = a[:, :, off : off + w]
        x[:, :, 2 * off + w : 2 * off + 2 * w] = b[:, :, off : off + w]
        off += w
    return x


def kernel(y_pred_logits: np.ndarray, y_true: np.ndarray, _trace=False) -> np.ndarray:
    nc = _get_nc()
    a = np.ascontiguousarray(y_pred_logits, dtype=np.float32).reshape(N_CORES, P, COLS)
    b = np.ascontiguousarray(y_true, dtype=np.float32).reshape(N_CORES, P, COLS)
    x = _pack(a, b)
    in_maps = [{"x": x[i]} for i in range(N_CORES)]
    # the fleet occasionally reports a transient NRT_EXEC_UNIT_UNRECOVERABLE
    # from a prior aborted run; it clears on retry
    last_err = None
    for attempt in range(3):
        try:
            r = run_bass_kernel_spmd(
                nc, in_maps, core_ids=list(range(N_CORES)), trace=_trace
            )
            break
        except Exception as exc:  # noqa: BLE001
            import traceback

            print(f"[kernel] attempt {attempt} failed: {exc!r}")
            traceback.print_exc()
            last_err = exc
            import time

            time.sleep(10.0)
    else:
        raise last_err
    per_sample = np.empty(N_CORES, dtype=np.float64)
    for i in range(N_CORES):
        row = np.asarray(r.results[i]["out"], dtype=np.float64).ravel()
        s2 = row[0:16].sum()
        sr = row[16:32].sum()
        s1 = row[32:48].sum()
        l2 = s2 / N_ELEM
        l1 = s1 / N_ELEM
        huber = 0.5 * (s2 - sr) / N_ELEM
        per_sample[i] = l2 if (l2 <= 1.0 or l2 < l1 * l1) else huber
    out = np.float32(per_sample.mean()).reshape(())
    if _trace:
        return out, r
    return out


# revision 18
# speedup vs baseline: 1.1449x; 1.1449x over previous
"""Adaptive Huber/MSE/L1 loss on 8 TRN2 NeuronCores (Bass/Tile), v8.

Reference math (per sample, N = 4,096,000 elements):
    e   = pred - true
    L2  = mean(e^2);  L1 = mean(|e|)
    huber = (S2 - SR) * 0.5 / N     (S2 = sum e^2, SR = sum relu(|e|-5)^2)
    use_l2 = (L2 <= 1) | (L2 < L1^2)
    loss = mean_over_batch(where(use_l2, L2, huber))

Sharding: data-parallel, sample i -> core i. The host interleaves pred
and true at DMA-tile granularity into ONE [128, 64000] f32 DRAM tensor
per core (one dma_start / one completion semaphore per tile). Each core
emits a [1,48] f32 row of partial sums in S2 | SR | S1 column regions;
the host sums the regions and finishes the branch math during unshard.

Measured facts driving this layout (v4-v7 HW traces, this session):
  - Stream runs at ~430 GB/s = ~98% of the 435 GB/s SBUF-AXI fabric
    ceiling; 32.77 MB arrives over [5.2us, 86.5us]. That is the floor.
  - DVE keeps a self-contained per-tile chain: sub (1.12ns/col, f32
    in), u16 |e| mask (0.34), max->m (0.34). v7 tried replacing the
    mask with ACT's Abs pass feeding DVE's max - the cross-engine
    round trip HOL-blocked DVE every pair and the stream collapsed to
    ~330 GB/s. Keep DVE dependencies DVE-local.
  - ACT passes cost ~0.975ns/col + 0.28us ACCUM_READ + ~0.18us gap per
    instruction; per-tile e^2+m^2 accums (5.18us per 2000-col tile)
    exceed the 4.84us arrival budget, so main-body Square passes are
    PAIRED across two tiles sharing one buffer (one pass + one read).
  - v6 lag analysis: DVE's m^2 mults on tiles {13,14,15,16} sat in its
    end-window and pushed the collapse matmul to 93.7us while PE idled
    at 50%. v8 keeps DVE m^2 (mult + PE row-sum chunks) only on early
    tiles {2,5,8,11} (chain closes t11, reduced mid-stream) and gives
    every later m^2 to ACT, whose tail queue (~9us) runs parallel to
    DVE's.
  - The ~250-instruction per-semaphore NEFF epilogue (~6us) and ~5us
    preamble are compiler-generated and fixed (queue-trim fails NRT).
    tensor_tensor_reduce and GpSimd tensor ops die at compile/NRT;
    DVE tensor_scalar rejects abs_max. All probed - don't reintroduce.
  - Pool ring slots chain WAR deps in ALLOCATION order: allocate
    work buffers in usage order or early tiles wait on later readers
    (measured: stream collapse at 80us).

Tail tiles shrink 1500/800/500/200; the 200-col last tile runs a short
DVE-local chain, ACT reduces the S1 PSUM chain via Identity+accum in
parallel, then one fp32 partition-collapse matmul and a 192 B output
DMA finish the kernel.
"""

import numpy as np

import concourse.bass as bass
import concourse.bacc as bacc
import concourse.mybir as mybir
from concourse.tile import TileContext
from concourse.bass_utils import run_bass_kernel_spmd

P = 128
COLS = 32000  # 160*160*160 / 128
DELTA = 5.0
N_CORES = 8
N_ELEM = float(P * COLS)
CHUNK = 500  # PE reduction column-chunk (PSUM bank limit 512 f32)
NF = 48  # fin columns: [0,20)=S2, [20,40)=SR, [40,48)=S1

F32 = mybir.dt.float32
U16 = mybir.dt.uint16
BF16 = mybir.dt.bfloat16
ALU = mybir.AluOpType
ACTF = mybir.ActivationFunctionType
AX = mybir.AxisListType

TILES = [2500, 2500] + [2000] * 12 + [1500, 800, 500, 200]
LAST = len(TILES) - 1  # t17: fully-DVE final tile
# e^2 ACT Square pass pairs (both tiles' |e| share one buffer)
E2_PAIRS = [(0, 1), (2, 3), (4, 5), (6, 7), (8, 9), (10, 11), (12, 13)]
E2_SINGLE = {14, 15, 16}
# m^2: DVE mult + PE chunks on early tiles only (chain closes t11);
# ACT pairs for the other main tiles, ACT singles for the tail
M2_DVE = {2, 5, 8, 11}
M2_PAIRS = [(0, 1), (3, 4), (6, 7), (9, 10), (12, 13)]
M2_SINGLE = {14, 15, 16}

# fin columns
S2_COL = {p: i for i, p in enumerate(E2_PAIRS)}  # pair -> col
S2_COL.update({14: 7, 15: 8, 16: 9, 17: 10})
SR_COL = {p: 20 + i for i, p in enumerate(M2_PAIRS)}
SR_COL.update({14: 25, 15: 26, 16: 27})
SR_D2 = 28
SR_T17 = 29
S1_COL = 40
S1_T17 = 41


def build():
    assert sum(TILES) == COLS
    e2_partner = {}
    for a, b in E2_PAIRS:
        e2_partner[a] = (a, b)
        e2_partner[b] = (a, b)
    m2_partner = {}
    for a, b in M2_PAIRS:
        m2_partner[a] = (a, b)
        m2_partner[b] = (a, b)

    mm_s1 = sum(
        (w + CHUNK - 1) // CHUNK for t, w in enumerate(TILES) if t != LAST
    )
    mm_d2 = sum((TILES[t] + CHUNK - 1) // CHUNK for t in M2_DVE)

    nc = bacc.Bacc(
        "TRN2",
        target_bir_lowering=False,
        debug=False,
        enable_asserts=False,
        num_devices=N_CORES,
    )
    x_ext = nc.dram_tensor("x", [P, 2 * COLS], F32, kind="ExternalInput")
    out_ext = nc.dram_tensor("out", [1, NF], F32, kind="ExternalOutput")

    with TileContext(nc) as tc:
        with (
            tc.tile_pool(name="iob", bufs=2) as iob_pool,
            tc.tile_pool(name="iom", bufs=5) as iom_pool,
            tc.tile_pool(name="work", bufs=3) as work_pool,
            tc.tile_pool(name="acc", bufs=1) as acc_pool,
            tc.tile_pool(name="psum", bufs=1, space="PSUM") as psum_pool,
        ):
            fin = acc_pool.tile([P, NF], F32)
            fin2 = acc_pool.tile([1, NF], F32)
            scr = acc_pool.tile([P, 5000], BF16)  # ACT pass output sink
            ones_bf = acc_pool.tile([P, 1], BF16)
            ones_f = acc_pool.tile([P, 1], F32)
            nc.vector.memset(ones_bf[:], 1.0)
            nc.vector.memset(ones_f[:], 1.0)
            nc.vector.memset(fin[:], 0.0)
            psum_s1 = psum_pool.tile([1, CHUNK], F32)  # S1 chain t0..16
            psum_d2 = psum_pool.tile([1, CHUNK], F32)  # m^2 chain, M2_DVE
            ps2 = psum_pool.tile([1, NF], F32)

            io_tiles = []
            col = 0
            for t, w in enumerate(TILES):
                pool = iob_pool if t < 2 else iom_pool
                xt = pool.tile([P, 2 * w], F32, tag="xb" if t < 2 else "xm")
                nc.sync.dma_start(out=xt[:], in_=x_ext[:, 2 * col : 2 * col + 2 * w])
                io_tiles.append(xt)
                col += w
            assert col == COLS

            # pair buffers: both members' |e| / m land in one tile so one
            # ACT Square pass + one ACCUM_READ covers the pair.
            # Allocated in USAGE order (see docstring).
            e_bufs = {}  # tile -> (buf, offset)
            m_bufs = {}

            mm_i = 0
            mmd_i = 0
            for t, w in enumerate(TILES):
                if t not in e_bufs:
                    if t in e2_partner:
                        a, b = e2_partner[t]
                        buf = work_pool.tile(
                            [P, TILES[a] + TILES[b]], BF16, tag="e",
                            name=f"ep{a}_{b}",
                        )
                        e_bufs[a] = (buf, 0)
                        e_bufs[b] = (buf, TILES[a])
                    else:
                        buf = work_pool.tile(
                            [P, w], BF16, tag="e", name=f"es{t}"
                        )
                        e_bufs[t] = (buf, 0)
                if t not in m_bufs:
                    if t in m2_partner:
                        a, b = m2_partner[t]
                        buf = work_pool.tile(
                            [P, TILES[a] + TILES[b]], BF16, tag="m",
                            name=f"mp{a}_{b}",
                        )
                        m_bufs[a] = (buf, 0)
                        m_bufs[b] = (buf, TILES[a])
                    else:
                        buf = work_pool.tile(
                            [P, w], BF16, tag="m", name=f"ms{t}"
                        )
                        m_bufs[t] = (buf, 0)
                xt = io_tiles[t]
                ebuf, eo = e_bufs[t]
                mbuf, mo = m_bufs[t]
                e = ebuf[:, eo : eo + w]
                m = mbuf[:, mo : mo + w]
                eu = ebuf.bitcast(U16)[:, eo : eo + w]
                # e = a - b (bf16 out: unbiased rounding, ~1e-5 rel err
                # on the final loss, far under the 2e-2 gate)
                nc.vector.tensor_tensor(e, xt[:, 0:w], xt[:, w : 2 * w], ALU.subtract)
                # |e| in place via u16 mask (2x 16-bit mode)
                nc.vector.tensor_scalar(eu, eu, 0x7FFF, None, ALU.bitwise_and)
                # m = max(|e|,5) - 5 == relu(|e|-5)
                nc.vector.tensor_scalar(m, e, DELTA, -DELTA, ALU.max, ALU.add)
                # S1: PE ones^T @ |e| chunks, one chain t0..16
                if t != LAST:
                    nch = (w + CHUNK - 1) // CHUNK
                    for c in range(nch):
                        cw = min(CHUNK, w - c * CHUNK)
                        nc.tensor.matmul(
                            psum_s1[0:1, 0:cw], ones_bf[:, 0:1],
                            ebuf[:, eo + c * CHUNK : eo + c * CHUNK + cw],
                            start=(mm_i == 0), stop=(mm_i == mm_s1 - 1),
                        )
                        mm_i += 1
                else:
                    nc.vector.tensor_reduce(
                        fin[:, S1_T17 : S1_T17 + 1], e, axis=AX.X,
                        op=ALU.add, apply_absolute_value=True,
                    )
                # m^2
                if t in M2_DVE:
                    nc.vector.tensor_tensor(m, m, m, ALU.mult)
                    nch = (w + CHUNK - 1) // CHUNK
                    for c in range(nch):
                        cw = min(CHUNK, w - c * CHUNK)
                        nc.tensor.matmul(
                            psum_d2[0:1, 0:cw], ones_bf[:, 0:1],
                            mbuf[:, mo + c * CHUNK : mo + c * CHUNK + cw],
                            start=(mmd_i == 0), stop=(mmd_i == mm_d2 - 1),
                        )
                        mmd_i += 1
                elif t in m2_partner:
                    pa, pb = m2_partner[t]
                    if t == pb:  # pair complete -> one ACT pass
                        pw = TILES[pa] + TILES[pb]
                        nc.scalar.activation(
                            scr[:, 0:pw], m_bufs[pa][0][:, 0:pw], ACTF.Square,
                            accum_out=fin[:, SR_COL[(pa, pb)] : SR_COL[(pa, pb)] + 1],
                        )
                elif t in M2_SINGLE:
                    nc.scalar.activation(
                        scr[:, 0:w], m, ACTF.Square,
                        accum_out=fin[:, SR_COL[t] : SR_COL[t] + 1],
                    )
                else:  # t == LAST
                    nc.vector.tensor_tensor(m, m, m, ALU.mult)
                    nc.vector.tensor_reduce(
                        fin[:, SR_T17 : SR_T17 + 1], m, axis=AX.X, op=ALU.add
                    )
                # e^2
                if t in e2_partner:
                    pa, pb = e2_partner[t]
                    if t == pb:
                        pw = TILES[pa] + TILES[pb]
                        nc.scalar.activation(
                            scr[:, 0:pw], e_bufs[pa][0][:, 0:pw], ACTF.Square,
                            accum_out=fin[:, S2_COL[(pa, pb)] : S2_COL[(pa, pb)] + 1],
                        )
                elif t in E2_SINGLE:
                    nc.scalar.activation(
                        scr[:, 0:w], e, ACTF.Square,
                        accum_out=fin[:, S2_COL[t] : S2_COL[t] + 1],
                    )
                else:  # t == LAST: square in place on DVE, plain reduce
                    nc.vector.tensor_tensor(e, e, e, ALU.mult)
                    nc.vector.tensor_reduce(
                        fin[:, S2_COL[t] : S2_COL[t] + 1], e, axis=AX.X, op=ALU.add
                    )
                # d2 chain closes at t11 -> reduce it mid-stream on ACT
                # (emitted after t13's ops; PE is long done by then)
                if t == 13:
                    nc.scalar.activation(
                        scr[0:1, 0:CHUNK], psum_d2[0:1, :], ACTF.Identity,
                        accum_out=fin[0:1, SR_D2 : SR_D2 + 1],
                    )
            assert mm_i == mm_s1 and mmd_i == mm_d2

            # S1 chain [1,500] reduce on ACT; chain closed at t16's PE
            # chunk, ACT reaches this after its tail singles - parallel
            # with DVE's t17 chain
            nc.scalar.activation(
                scr[0:1, 0:CHUNK], psum_s1[0:1, :], ACTF.Identity,
                accum_out=fin[0:1, S1_COL : S1_COL + 1],
            )

            # partition-collapse so the output is one 192 B DMA packet
            nc.tensor.matmul(ps2[0:1, 0:NF], ones_f[:, 0:1], fin[:, 0:NF],
                             start=True, stop=True)
            nc.vector.tensor_scalar(fin2[:], ps2[0:1, 0:NF], 1.0, None, ALU.mult)
            nc.sync.dma_start(out=out_ext[:, :], in_=fin2[:])

    nc.compile()
    return nc


_NC_CACHE = {}


def _get_nc():
    if "nc" not in _NC_CACHE:
        _NC_CACHE["nc"] = build()
    return _NC_CACHE["nc"]


def _pack(a: np.ndarray, b: np.ndarray) -> np.ndarray:
    """Interleave pred/true at DMA-tile granularity: one [P, 2*COLS]
    tensor per core, tile t occupying cols [2*off, 2*off+2*w) with the
    pred block first and the true block second."""
    x = np.empty((N_CORES, P, 2 * COLS), dtype=np.float32)
    off = 0
    for w in TILES:
        x[:, :, 2 * off : 2 * off + w] = a[:, :, off : off + w]
        x[:, :, 2 * off + w : 2 * off + 2 * w] = b[:, :, off : off + w]
        off += w
    return x


def kernel(y_pred_logits: np.ndarray, y_true: np.ndarray, _trace=False) -> np.ndarray:
    nc = _get_nc()
    a = np.ascontiguousarray(y_pred_logits, dtype=np.float32).reshape(N_CORES, P, COLS)
    b = np.ascontiguousarray(y_true, dtype=np.float32).reshape(N_CORES, P, COLS)
    x = _pack(a, b)
    in_maps = [{"x": x[i]} for i in range(N_CORES)]
    # the fleet occasionally reports a transient NRT_EXEC_UNIT_UNRECOVERABLE
    # from a prior aborted run; it clears on retry
    last_err = None
    for attempt in range(3):
        try:
            r = run_bass_kernel_spmd(
                nc, in_maps, core_ids=list(range(N_CORES)), trace=_trace
            )
            break
        except Exception as exc:  # noqa: BLE001
            import traceback

            print(f"[kernel] attempt {attempt} failed: {exc!r}")
            traceback.print_exc()
            last_err = exc
            import time

            time.sleep(10.0)
    else:
        raise last_err
    per_sample = np.empty(N_CORES, dtype=np.float64)
    for i in range(N_CORES):
        row = np.asarray(r.results[i]["out"], dtype=np.float64).ravel()
        s2 = row[0:20].sum()
        sr = row[20:40].sum()
        s1 = row[40:48].sum()
        l2 = s2 / N_ELEM
        l1 = s1 / N_ELEM
        huber = 0.5 * (s2 - sr) / N_ELEM
        per_sample[i] = l2 if (l2 <= 1.0 or l2 < l1 * l1) else huber
    out = np.float32(per_sample.mean()).reshape(())
    if _trace:
        return out, r
    return out
